# revision 8
# baseline (speedup 1.0000x reference)
"""Trainium2 Bass kernel for ConvolutionalSelfAttention.

Math (per batch image, fp32):
  X [256, 64] pixels.  For each 3x3 window n (196 of them) and local slot k
  (9), the reference softmax-attends over the 247 pixels outside window n
  with logits TEMP*cos(x_g, x_{pix(n,k)}), weights s_g = x_g @ Wg + bg, and
  aggregates the window pixels with the resulting per-slot weights.

  Key factorization: all needed cosine sims live in one 256x256 gram
  E = exp(TEMP * Xn @ Xn.T); window/global masking is linear, so
      D[p, n] = sum_g maskg[g, n] * E[g, p]          (denominator)
      N[p, n] = sum_g maskg[g, n] * s'_g * E[g, p]   (numerator)
      A[p, n] = maskl[p, n] * N[p, n] / D[p, n]
      out[n, c] = sum_p A[p, n] * X[p, c]
  -> everything is dense matmuls + one exp, no per-window gathers.

Sharding: data-parallel over batch; 32 images / 8 cores = 4 images per core.

Host<->device transport over the axon tunnel dominates wall time (~45 MB/s,
~7 ms/op), so the kernel ships the bare minimum: x as bf16 and a 65-float
wg||bg vector per core; y returns as bf16.  The window masks and the
transpose identity are generated on device (affine_select iotas + two tiny
matmuls exploiting maskl[p,n] = winm[p//16, n//14] * winm[p%16, n%14]).
A persistent JAX compilation cache avoids re-running neuronx-cc on the
fresh jit closure run_bass_kernel_spmd builds per call.
"""

import sys
import numpy as np
import ml_dtypes

sys.path.insert(0, "/opt/trn_rl_repo")

from contextlib import ExitStack

import jax

for _k, _v in [
    ("jax_compilation_cache_dir", "/tmp/jax_pcc"),
    ("jax_persistent_cache_min_entry_size_bytes", -1),
    ("jax_persistent_cache_min_compile_time_secs", 0.0),
    ("jax_persistent_cache_enable_xla_caches", "all"),
]:
    try:
        jax.config.update(_k, _v)
    except Exception:
        pass

import concourse.bass as bass
import concourse.bacc as bacc
import concourse.tile as tile
from concourse import bass2jax as _b2j
from concourse import mybir
from concourse.bass_utils import run_bass_kernel_spmd

H = 16
W = 16
C = 64
K = 3
B = 32
CH = H - K + 1
CW = W - K + 1
NC = CH * CW          # 196
HW = H * W            # 256
TEMP = 10.0
NCORES = 8
BPC = B // NCORES     # 4 images per core
P = 128

F32 = mybir.dt.float32
BF16 = mybir.dt.bfloat16
AF = mybir.ActivationFunctionType
ALU = mybir.AluOpType


def _bcast_ap(ap, parts):
    """[*dims] -> [parts, *dims] with partition stride 0 (DMA broadcast)."""
    return bass.AP(tensor=ap.tensor, offset=ap.offset, ap=[[0, parts]] + list(ap.ap))


def _patch_act_tables():
    """Steer every Ln/Exp activation to `natural_log_exp_and_others` so the
    kernel needs exactly one ACT table load instead of thrashing between the
    Ln-only and Exp-only sets (~2.7us per switch)."""
    from concourse import hw_specs
    orig_fn = hw_specs.get_activation_tables.__wrapped__

    def patched(arch):
        tabs = dict(orig_fn(arch))
        if "natural_log_exp_and_others" in tabs:
            for name in tabs:
                if name != "natural_log_exp_and_others":
                    tabs[name] = tabs[name] - {AF.Ln, AF.Exp}
        return tabs

    bacc.get_activation_tables = patched


def _gen_consts(nc, consts, pg_pool):
    """Generate the 128x128 transpose identity and the window masks on
    device.  maskl[16*pr+pc, 14*nr+nj] = winm[pr,nr]*winm[pc,nj] with
    winm[r,c] = 1 iff 0 <= r-c <= 2; the p//16 / p%16 selectors come from
    affine iotas and two 16-contract matmuls expand the Kronecker factors."""
    ident = consts.tile([P, P], F32, tag="ident")
    nc.gpsimd.memset(ident, 1.0)
    nc.gpsimd.affine_select(out=ident, in_=ident, compare_op=ALU.is_equal,
                            fill=0.0, base=0, pattern=[[-1, P]],
                            channel_multiplier=1)

    # st[k, p] = 1 iff p//16 == k ; tt[k, p] = 1 iff p%16 == k
    st = consts.tile([16, HW], F32, tag="st")
    nc.gpsimd.memset(st, 1.0)
    nc.gpsimd.affine_select(out=st, in_=st, compare_op=ALU.is_equal, fill=0.0,
                            base=0, pattern=[[1, 16], [0, 16]],
                            channel_multiplier=-1)
    tt = consts.tile([16, HW], F32, tag="tt")
    nc.gpsimd.memset(tt, 1.0)
    nc.gpsimd.affine_select(out=tt, in_=tt, compare_op=ALU.is_equal, fill=0.0,
                            base=0, pattern=[[0, 16], [1, 16]],
                            channel_multiplier=-1)
    winm = consts.tile([16, CW], F32, tag="winm")
    nc.gpsimd.memset(winm, 1.0)
    nc.gpsimd.affine_select(out=winm, in_=winm, compare_op=ALU.is_ge, fill=0.0,
                            base=0, pattern=[[-1, CW]], channel_multiplier=1)
    # r-c <= 2 as -(r-c)+2 >= 0 (is_le is unimplemented in walrus codegen)
    nc.gpsimd.affine_select(out=winm, in_=winm, compare_op=ALU.is_ge, fill=0.0,
                            base=2, pattern=[[1, CW]], channel_multiplier=-1)

    mg, ml = [], []
    scratch = pg_pool.tile([P, HW], F32, tag="g0")
    for t in range(2):
        rp = scratch[:, t * 2 * CW:t * 2 * CW + CW]
        nc.tensor.matmul(out=rp, lhsT=st[:, t * P:(t + 1) * P], rhs=winm,
                         start=True, stop=True)
        rt = consts.tile([P, CW], F32, tag=f"rt{t}")
        nc.vector.tensor_copy(out=rt, in_=rp)
        cp = scratch[:, (t * 2 + 1) * CW:(t * 2 + 2) * CW]
        nc.tensor.matmul(out=cp, lhsT=tt[:, t * P:(t + 1) * P], rhs=winm,
                         start=True, stop=True)
        ct = consts.tile([P, CW], F32, tag=f"ct{t}")
        nc.vector.tensor_copy(out=ct, in_=cp)
        mlt = consts.tile([P, NC], BF16, tag=f"ml{t}")
        for nr in range(CH):
            nc.vector.tensor_scalar_mul(out=mlt[:, nr * CW:(nr + 1) * CW],
                                        in0=ct, scalar1=rt[:, nr:nr + 1])
        ml.append(mlt)
        ones = consts.tile([P, NC], BF16, tag=f"ones{t}")
        nc.gpsimd.memset(ones, 1.0)
        mgt = consts.tile([P, NC], BF16, tag=f"mg{t}")
        nc.vector.tensor_sub(out=mgt, in0=ones, in1=mlt)
        mg.append(mgt)
    return ident, mg, ml


def build_bass():
    _patch_act_tables()
    nc = bacc.Bacc("TRN2", target_bir_lowering=False, debug=False)

    x = nc.declare_dram_parameter("x", [BPC, HW, C], BF16, isOutput=False)
    wgb = nc.declare_dram_parameter("wgb", [C + 1], F32, isOutput=False)
    y = nc.declare_dram_parameter("y", [BPC, NC, C], BF16, isOutput=True)

    with ExitStack() as ctx:
        tc = ctx.enter_context(tile.TileContext(nc))
        consts = ctx.enter_context(tc.tile_pool(name="consts", bufs=1))
        sb = ctx.enter_context(tc.tile_pool(name="sb", bufs=1))
        pt_pool = ctx.enter_context(tc.tile_pool(name="pt", bufs=1, space="PSUM"))
        pg_pool = ctx.enter_context(tc.tile_pool(name="pg", bufs=1, space="PSUM"))
        pnd_pool = ctx.enter_context(tc.tile_pool(name="pnd", bufs=1, space="PSUM"))

        ident, mg, ml = _gen_consts(nc, consts, pg_pool)

        wb = consts.tile([P, C], F32, tag="wb")
        nc.sync.dma_start(out=wb, in_=_bcast_ap(wgb[0:C], P))
        bgb = consts.tile([P, 1], F32, tag="bgb")
        nc.sync.dma_start(out=bgb, in_=_bcast_ap(wgb[C:C + 1], P))

        # ---- stage 1: load, row stats; ACT does only Ln here ----
        xt = [[None] * 2 for _ in range(BPC)]
        sp = [[None] * 2 for _ in range(BPC)]
        uu = [[None] * 2 for _ in range(BPC)]
        for b in range(BPC):
            for t in range(2):
                xbt = sb.tile([P, C], BF16, tag=f"xb{b}{t}")
                nc.sync.dma_start(out=xbt, in_=x[b, t * P:(t + 1) * P, :])
                xtt = sb.tile([P, C], F32, tag=f"x{b}{t}")
                nc.vector.tensor_copy(out=xtt, in_=xbt)
                xt[b][t] = xtt
                scr = sb.tile([P, C], F32, tag=f"scr{b}{t}")
                nc.gpsimd.tensor_mul(out=scr, in0=xtt, in1=xtt)
                ss = sb.tile([P, 1], F32, tag=f"ss{b}{t}")
                nc.vector.reduce_sum(out=ss, in_=scr, axis=mybir.AxisListType.X)
                scr2 = sb.tile([P, C], F32, tag=f"scr2{b}{t}")
                nc.gpsimd.tensor_mul(out=scr2, in0=xtt, in1=wb)
                s0 = sb.tile([P, 1], F32, tag=f"s0{b}{t}")
                nc.vector.reduce_sum(out=s0, in_=scr2, axis=mybir.AxisListType.X)
                spt = sb.tile([P, 1], F32, tag=f"sp{b}{t}")
                nc.vector.tensor_scalar_add(out=spt, in0=s0, scalar1=bgb[:, 0:1])
                sp[b][t] = spt
                u = sb.tile([P, 1], F32, tag=f"u{b}{t}")
                nc.scalar.activation(out=u, in_=ss, func=AF.Ln)
                uu[b][t] = u

        # ---- stage 2: normalize, transpose, gram, E = exp ----
        e = [[None] * 2 for _ in range(BPC)]
        for b in range(BPC):
            xn = []
            for t in range(2):
                rn = sb.tile([P, 1], F32, tag=f"rn{b}{t}")
                nc.scalar.activation(out=rn, in_=uu[b][t], func=AF.Exp, scale=-0.5)
                xnt = sb.tile([P, C], F32, tag=f"xn{b}{t}")
                nc.vector.tensor_scalar_mul(out=xnt, in0=xt[b][t], scalar1=rn)
                xn.append(xnt)
            xnT = sb.tile([C, HW], F32, tag=f"xnT{b}")
            for t in range(2):
                tp = pt_pool.tile([C, P], F32, tag=f"tp{t}")
                nc.tensor.transpose(out=tp, in_=xn[t], identity=ident)
                nc.vector.tensor_copy(out=xnT[:, t * P:(t + 1) * P], in_=tp)
            for t in range(2):
                g = pg_pool.tile([P, HW], F32, tag=f"g{t}")
                nc.tensor.matmul(
                    out=g, lhsT=xnT[:, t * P:(t + 1) * P], rhs=xnT,
                    start=True, stop=True)
                et = sb.tile([P, HW], BF16, tag=f"e{b}{t}")
                nc.scalar.activation(out=et, in_=g, func=AF.Exp, scale=TEMP)
                e[b][t] = et

        # ---- stage 3: N/D matmuls (bf16 in, f32 psum); ACT: Ln(D) ----
        u2 = [[None] * 2 for _ in range(BPC)]
        nps = [[None] * 2 for _ in range(BPC)]
        for b in range(BPC):
            ms = []
            for t in range(2):
                mst = sb.tile([P, NC], BF16, tag=f"ms{b}{t}")
                nc.vector.tensor_scalar_mul(out=mst, in0=mg[t], scalar1=sp[b][t])
                ms.append(mst)
            for pti in range(2):
                psl = slice(pti * P, (pti + 1) * P)
                d_ps = pnd_pool.tile([P, NC], F32, tag=f"d{pti}")
                nc.tensor.matmul(out=d_ps, lhsT=e[b][0][:, psl], rhs=mg[0],
                                 start=True, stop=False)
                nc.tensor.matmul(out=d_ps, lhsT=e[b][1][:, psl], rhs=mg[1],
                                 start=False, stop=True)
                n_ps = pnd_pool.tile([P, NC], F32, tag=f"n{pti}")
                nc.tensor.matmul(out=n_ps, lhsT=e[b][0][:, psl], rhs=ms[0],
                                 start=True, stop=False)
                nc.tensor.matmul(out=n_ps, lhsT=e[b][1][:, psl], rhs=ms[1],
                                 start=False, stop=True)
                u2t = sb.tile([P, NC], F32, tag=f"u2{b}{pti}")
                nc.scalar.activation(out=u2t, in_=d_ps, func=AF.Ln)
                u2[b][pti] = u2t
                nsb = sb.tile([P, NC], F32, tag=f"nsb{b}{pti}")
                nc.vector.tensor_copy(out=nsb, in_=n_ps)
                nps[b][pti] = nsb

        # ---- stage 4: A = maskl * N * exp(-lnD); out = A.T @ X ----
        for b in range(BPC):
            a = []
            for pti in range(2):
                rd = sb.tile([P, NC], F32, tag=f"rd{b}{pti}")
                nc.scalar.activation(out=rd, in_=u2[b][pti], func=AF.Exp,
                                     scale=-1.0)
                a1 = sb.tile([P, NC], F32, tag=f"a1{b}{pti}")
                nc.vector.tensor_mul(out=a1, in0=nps[b][pti], in1=rd)
                a2 = sb.tile([P, NC], F32, tag=f"a2{b}{pti}")
                nc.gpsimd.tensor_mul(out=a2, in0=a1, in1=ml[pti])
                a.append(a2)
            for nt, (n0, nsz) in enumerate(((0, P), (P, NC - P))):
                o = pg_pool.tile([P, C], F32, tag=f"g{nt}")
                nc.tensor.matmul(out=o[:nsz, :], lhsT=a[0][:, n0:n0 + nsz],
                                 rhs=xt[b][0], start=True, stop=False)
                nc.tensor.matmul(out=o[:nsz, :], lhsT=a[1][:, n0:n0 + nsz],
                                 rhs=xt[b][1], start=False, stop=True)
                osb = sb.tile([P, C], BF16, tag=f"osb{b}{nt}")
                nc.vector.tensor_copy(out=osb[:nsz, :], in_=o[:nsz, :])
                nc.sync.dma_start(out=y[b, n0:n0 + nsz, :], in_=osb[:nsz, :])

    nc.compile()
    return nc


_ORIG_RUN_VIA_PJRT = _b2j.run_bass_via_pjrt
_FAST_CACHE = {}


def _build_fast_entry(nc, n_cores):
    """Reusable jitted executor for ``nc``.

    ``run_bass_via_pjrt`` builds a fresh jit closure per call, so every call
    re-traces, re-lowers, reloads the executable, and ships an extra zeroed
    donation buffer per output.  This kernel writes every element of its
    single output, so the zero-donation is unnecessary; caching the jitted
    callable turns repeat calls into pure transfer+execute."""
    from jax.experimental.shard_map import shard_map
    from jax.sharding import Mesh, PartitionSpec

    _b2j.install_neuronx_cc_hook()
    assert nc.dbg_addr is None and not nc.dbg_callbacks

    partition_name = nc.partition_id_tensor.name if nc.partition_id_tensor else None
    in_names = []
    out_names = []
    out_avals = []
    for alloc in nc.m.functions[0].allocations:
        if not isinstance(alloc, mybir.MemoryLocationSet):
            continue
        name = alloc.memorylocations[0].name
        if alloc.kind == "ExternalInput":
            if name != partition_name:
                in_names.append(name)
        elif alloc.kind == "ExternalOutput":
            out_names.append(name)
            out_avals.append(jax.core.ShapedArray(
                tuple(alloc.tensor_shape), mybir.dt.np(alloc.dtype)))
    n_params = len(in_names)
    bind_names = tuple(in_names) + ((partition_name,) if partition_name else ())

    def _body(*args):
        operands = list(args)
        if partition_name is not None:
            operands.append(_b2j.partition_id_tensor())
        return tuple(_b2j._bass_exec_p.bind(
            *operands,
            out_avals=tuple(out_avals),
            in_names=bind_names,
            out_names=tuple(out_names),
            lowering_input_output_aliases=(),
            sim_require_finite=True,
            sim_require_nnan=True,
            nc=nc,
        ))

    devices = jax.devices()[:n_cores]
    assert len(devices) == n_cores
    mesh = Mesh(np.asarray(devices), ("core",))
    fn = jax.jit(
        shard_map(_body, mesh=mesh,
                  in_specs=(PartitionSpec("core"),) * n_params,
                  out_specs=(PartitionSpec("core"),) * len(out_names),
                  check_rep=False),
        keep_unused=True,
    )

    def run(in_maps):
        per_core = [[np.asarray(m[name]) for name in in_names] for m in in_maps]
        concat = [
            np.concatenate([per_core[c][i] for c in range(n_cores)], axis=0)
            for i in range(n_params)
        ]
        out_arrs = fn(*concat)
        outs_np = [np.asarray(a) for a in out_arrs]
        return [
            {name: outs_np[i].reshape(n_cores, *out_avals[i].shape)[c]
             for i, name in enumerate(out_names)}
            for c in range(n_cores)
        ]

    return run


def _fast_run_via_pjrt(nc, in_maps, n_cores):
    try:
        run = _FAST_CACHE.get((id(nc), n_cores))
        if run is None:
            run = _build_fast_entry(nc, n_cores)
            _FAST_CACHE[(id(nc), n_cores)] = run
        return run(in_maps)
    except Exception:
        return _ORIG_RUN_VIA_PJRT(nc, in_maps, n_cores)


if _b2j.run_bass_via_pjrt is _ORIG_RUN_VIA_PJRT:
    _b2j.run_bass_via_pjrt = _fast_run_via_pjrt


_NC_CACHE = None


def _get_nc():
    global _NC_CACHE
    if _NC_CACHE is None:
        _NC_CACHE = build_bass()
    return _NC_CACHE


def build_in_maps(batch: np.ndarray, Wg: np.ndarray, bg: np.ndarray):
    X = np.ascontiguousarray(
        np.asarray(batch).reshape(B, HW, C).astype(ml_dtypes.bfloat16))
    wgb = np.concatenate([
        np.asarray(Wg, np.float32).reshape(C), np.asarray(bg, np.float32)])
    return [
        {"x": X[c * BPC:(c + 1) * BPC], "wgb": wgb}
        for c in range(NCORES)
    ]


def kernel(batch: np.ndarray, Wg: np.ndarray, bg: np.ndarray) -> np.ndarray:
    nc = _get_nc()
    in_maps = build_in_maps(batch, Wg, bg)
    res = run_bass_kernel_spmd(nc, in_maps, list(range(NCORES)))
    out = np.concatenate([np.asarray(res.results[c]["y"]) for c in range(NCORES)], 0)
    return out.reshape(B, CH, CW, C).astype(np.float32)


# revision 9
# speedup vs baseline: 1.4688x; 1.4688x over previous
"""Trainium2 Bass kernel for ConvolutionalSelfAttention.

Math (per batch image, fp32):
  X [256, 64] pixels.  For each 3x3 window n (196 of them) and local slot k
  (9), the reference softmax-attends over the 247 pixels outside window n
  with logits TEMP*cos(x_g, x_{pix(n,k)}), weights s_g = x_g @ Wg + bg, and
  aggregates the window pixels with the resulting per-slot weights.

  Key factorization: all needed cosine sims live in one 256x256 gram
  E = exp(TEMP * Xn @ Xn.T); window/global masking is linear, so
      D[p, n] = sum_g maskg[g, n] * E[g, p]          (denominator)
      N[p, n] = sum_g maskg[g, n] * s'_g * E[g, p]   (numerator)
      A[p, n] = maskl[p, n] * N[p, n] / D[p, n]
      out[n, c] = sum_p A[p, n] * X[p, c]
  -> everything is dense matmuls + one exp, no per-window gathers.

Sharding: data-parallel over batch; 32 images / 8 cores = 4 images per core.

Host<->device transport over the axon tunnel dominates wall time (~45 MB/s,
~7 ms/op), so the kernel ships the bare minimum: x as bf16 and a 65-float
wg||bg vector per core; y returns as bf16.  The window masks and the
transpose identity are generated on device (affine_select iotas + two tiny
matmuls exploiting maskl[p,n] = winm[p//16, n//14] * winm[p%16, n%14]).
A persistent JAX compilation cache avoids re-running neuronx-cc on the
fresh jit closure run_bass_kernel_spmd builds per call, and
run_bass_via_pjrt is swapped for a cached equivalent (fallback to stock on
any error) that skips per-call re-trace/re-lower/executable-reload and the
zeroed donation upload (this kernel writes every output element).
"""

import sys
import numpy as np
import ml_dtypes

sys.path.insert(0, "/opt/trn_rl_repo")

from contextlib import ExitStack

import jax

for _k, _v in [
    ("jax_compilation_cache_dir", "/tmp/jax_pcc"),
    ("jax_persistent_cache_min_entry_size_bytes", -1),
    ("jax_persistent_cache_min_compile_time_secs", 0.0),
    ("jax_persistent_cache_enable_xla_caches", "all"),
]:
    try:
        jax.config.update(_k, _v)
    except Exception:
        pass

import concourse.bass as bass
import concourse.bacc as bacc
import concourse.tile as tile
from concourse import bass2jax as _b2j
from concourse import mybir
from concourse.bass_utils import run_bass_kernel_spmd

H = 16
W = 16
C = 64
K = 3
B = 32
CH = H - K + 1
CW = W - K + 1
NC = CH * CW          # 196
HW = H * W            # 256
TEMP = 10.0
NCORES = 8
BPC = B // NCORES     # 4 images per core
P = 128

F32 = mybir.dt.float32
BF16 = mybir.dt.bfloat16
AF = mybir.ActivationFunctionType
ALU = mybir.AluOpType


def _bcast_ap(ap, parts):
    """[*dims] -> [parts, *dims] with partition stride 0 (DMA broadcast)."""
    return bass.AP(tensor=ap.tensor, offset=ap.offset, ap=[[0, parts]] + list(ap.ap))


def _patch_act_tables():
    """Steer every Ln/Exp activation to `natural_log_exp_and_others` so the
    kernel needs exactly one ACT table load instead of thrashing between the
    Ln-only and Exp-only sets (~2.7us per switch)."""
    from concourse import hw_specs
    orig_fn = hw_specs.get_activation_tables.__wrapped__

    def patched(arch):
        tabs = dict(orig_fn(arch))
        if "natural_log_exp_and_others" in tabs:
            for name in tabs:
                if name != "natural_log_exp_and_others":
                    tabs[name] = tabs[name] - {AF.Ln, AF.Exp}
        return tabs

    bacc.get_activation_tables = patched


def _gen_consts(nc, consts, pg_pool):
    """Generate the 128x128 transpose identity and the window masks on
    device.  maskl[16*pr+pc, 14*nr+nj] = winm[pr,nr]*winm[pc,nj] with
    winm[r,c] = 1 iff 0 <= r-c <= 2; the p//16 / p%16 selectors come from
    affine iotas and two 16-contract matmuls expand the Kronecker factors."""
    ident = consts.tile([P, P], F32, tag="ident")
    nc.gpsimd.memset(ident, 1.0)
    nc.gpsimd.affine_select(out=ident, in_=ident, compare_op=ALU.is_equal,
                            fill=0.0, base=0, pattern=[[-1, P]],
                            channel_multiplier=1)

    # st[k, p] = 1 iff p//16 == k ; tt[k, p] = 1 iff p%16 == k
    st = consts.tile([16, HW], F32, tag="st")
    nc.gpsimd.memset(st, 1.0)
    nc.gpsimd.affine_select(out=st, in_=st, compare_op=ALU.is_equal, fill=0.0,
                            base=0, pattern=[[1, 16], [0, 16]],
                            channel_multiplier=-1)
    tt = consts.tile([16, HW], F32, tag="tt")
    nc.gpsimd.memset(tt, 1.0)
    nc.gpsimd.affine_select(out=tt, in_=tt, compare_op=ALU.is_equal, fill=0.0,
                            base=0, pattern=[[0, 16], [1, 16]],
                            channel_multiplier=-1)
    winm = consts.tile([16, CW], F32, tag="winm")
    nc.gpsimd.memset(winm, 1.0)
    nc.gpsimd.affine_select(out=winm, in_=winm, compare_op=ALU.is_ge, fill=0.0,
                            base=0, pattern=[[-1, CW]], channel_multiplier=1)
    # r-c <= 2 as -(r-c)+2 >= 0 (is_le is unimplemented in walrus codegen)
    nc.gpsimd.affine_select(out=winm, in_=winm, compare_op=ALU.is_ge, fill=0.0,
                            base=2, pattern=[[1, CW]], channel_multiplier=-1)

    mg, ml = [], []
    scratch = pg_pool.tile([P, HW], F32, tag="g0")
    for t in range(2):
        rp = scratch[:, t * 2 * CW:t * 2 * CW + CW]
        nc.tensor.matmul(out=rp, lhsT=st[:, t * P:(t + 1) * P], rhs=winm,
                         start=True, stop=True)
        rt = consts.tile([P, CW], F32, tag=f"rt{t}")
        nc.vector.tensor_copy(out=rt, in_=rp)
        cp = scratch[:, (t * 2 + 1) * CW:(t * 2 + 2) * CW]
        nc.tensor.matmul(out=cp, lhsT=tt[:, t * P:(t + 1) * P], rhs=winm,
                         start=True, stop=True)
        ct = consts.tile([P, CW], F32, tag=f"ct{t}")
        nc.vector.tensor_copy(out=ct, in_=cp)
        mlt = consts.tile([P, NC], BF16, tag=f"ml{t}")
        for nr in range(CH):
            nc.vector.tensor_scalar_mul(out=mlt[:, nr * CW:(nr + 1) * CW],
                                        in0=ct, scalar1=rt[:, nr:nr + 1])
        ml.append(mlt)
        ones = consts.tile([P, NC], BF16, tag=f"ones{t}")
        nc.gpsimd.memset(ones, 1.0)
        mgt = consts.tile([P, NC], BF16, tag=f"mg{t}")
        nc.vector.tensor_sub(out=mgt, in0=ones, in1=mlt)
        mg.append(mgt)
    return ident, mg, ml


def build_bass():
    _patch_act_tables()
    nc = bacc.Bacc("TRN2", target_bir_lowering=False, debug=False)

    x = nc.declare_dram_parameter("x", [BPC, HW, C], BF16, isOutput=False)
    wgb = nc.declare_dram_parameter("wgb", [C + 1], F32, isOutput=False)
    y = nc.declare_dram_parameter("y", [BPC, NC, C], BF16, isOutput=True)

    with ExitStack() as ctx:
        tc = ctx.enter_context(tile.TileContext(nc))
        consts = ctx.enter_context(tc.tile_pool(name="consts", bufs=1))
        sb = ctx.enter_context(tc.tile_pool(name="sb", bufs=1))
        pt_pool = ctx.enter_context(tc.tile_pool(name="pt", bufs=1, space="PSUM"))
        pg_pool = ctx.enter_context(tc.tile_pool(name="pg", bufs=1, space="PSUM"))
        pnd_pool = ctx.enter_context(tc.tile_pool(name="pnd", bufs=1, space="PSUM"))

        ident, mg, ml = _gen_consts(nc, consts, pg_pool)

        wb = consts.tile([P, C], F32, tag="wb")
        nc.sync.dma_start(out=wb, in_=_bcast_ap(wgb[0:C], P))
        bgb = consts.tile([P, 1], F32, tag="bgb")
        nc.sync.dma_start(out=bgb, in_=_bcast_ap(wgb[C:C + 1], P))

        # ---- stage 1: load, row stats; ACT does only Ln here ----
        xt = [[None] * 2 for _ in range(BPC)]
        sp = [[None] * 2 for _ in range(BPC)]
        uu = [[None] * 2 for _ in range(BPC)]
        for b in range(BPC):
            for t in range(2):
                xbt = sb.tile([P, C], BF16, tag=f"xb{b}{t}")
                nc.sync.dma_start(out=xbt, in_=x[b, t * P:(t + 1) * P, :])
                xtt = sb.tile([P, C], F32, tag=f"x{b}{t}")
                nc.vector.tensor_copy(out=xtt, in_=xbt)
                xt[b][t] = xtt
                scr = sb.tile([P, C], F32, tag=f"scr{b}{t}")
                nc.gpsimd.tensor_mul(out=scr, in0=xtt, in1=xtt)
                ss = sb.tile([P, 1], F32, tag=f"ss{b}{t}")
                nc.vector.reduce_sum(out=ss, in_=scr, axis=mybir.AxisListType.X)
                scr2 = sb.tile([P, C], F32, tag=f"scr2{b}{t}")
                nc.gpsimd.tensor_mul(out=scr2, in0=xtt, in1=wb)
                s0 = sb.tile([P, 1], F32, tag=f"s0{b}{t}")
                nc.vector.reduce_sum(out=s0, in_=scr2, axis=mybir.AxisListType.X)
                spt = sb.tile([P, 1], F32, tag=f"sp{b}{t}")
                nc.vector.tensor_scalar_add(out=spt, in0=s0, scalar1=bgb[:, 0:1])
                sp[b][t] = spt
                u = sb.tile([P, 1], F32, tag=f"u{b}{t}")
                nc.scalar.activation(out=u, in_=ss, func=AF.Ln)
                uu[b][t] = u

        # ---- stage 2: normalize, transpose, gram, E = exp ----
        e = [[None] * 2 for _ in range(BPC)]
        for b in range(BPC):
            xn = []
            for t in range(2):
                rn = sb.tile([P, 1], F32, tag=f"rn{b}{t}")
                nc.scalar.activation(out=rn, in_=uu[b][t], func=AF.Exp, scale=-0.5)
                xnt = sb.tile([P, C], F32, tag=f"xn{b}{t}")
                nc.vector.tensor_scalar_mul(out=xnt, in0=xt[b][t], scalar1=rn)
                xn.append(xnt)
            xnT = sb.tile([C, HW], F32, tag=f"xnT{b}")
            for t in range(2):
                tp = pt_pool.tile([C, P], F32, tag=f"tp{t}")
                nc.tensor.transpose(out=tp, in_=xn[t], identity=ident)
                nc.vector.tensor_copy(out=xnT[:, t * P:(t + 1) * P], in_=tp)
            for t in range(2):
                g = pg_pool.tile([P, HW], F32, tag=f"g{t}")
                nc.tensor.matmul(
                    out=g, lhsT=xnT[:, t * P:(t + 1) * P], rhs=xnT,
                    start=True, stop=True)
                et = sb.tile([P, HW], BF16, tag=f"e{b}{t}")
                nc.scalar.activation(out=et, in_=g, func=AF.Exp, scale=TEMP)
                e[b][t] = et

        # ---- stage 3: N/D matmuls (bf16 in, f32 psum); ACT: Ln(D) ----
        u2 = [[None] * 2 for _ in range(BPC)]
        nps = [[None] * 2 for _ in range(BPC)]
        for b in range(BPC):
            ms = []
            for t in range(2):
                mst = sb.tile([P, NC], BF16, tag=f"ms{b}{t}")
                nc.vector.tensor_scalar_mul(out=mst, in0=mg[t], scalar1=sp[b][t])
                ms.append(mst)
            for pti in range(2):
                psl = slice(pti * P, (pti + 1) * P)
                d_ps = pnd_pool.tile([P, NC], F32, tag=f"d{pti}")
                nc.tensor.matmul(out=d_ps, lhsT=e[b][0][:, psl], rhs=mg[0],
                                 start=True, stop=False)
                nc.tensor.matmul(out=d_ps, lhsT=e[b][1][:, psl], rhs=mg[1],
                                 start=False, stop=True)
                n_ps = pnd_pool.tile([P, NC], F32, tag=f"n{pti}")
                nc.tensor.matmul(out=n_ps, lhsT=e[b][0][:, psl], rhs=ms[0],
                                 start=True, stop=False)
                nc.tensor.matmul(out=n_ps, lhsT=e[b][1][:, psl], rhs=ms[1],
                                 start=False, stop=True)
                u2t = sb.tile([P, NC], F32, tag=f"u2{b}{pti}")
                nc.scalar.activation(out=u2t, in_=d_ps, func=AF.Ln)
                u2[b][pti] = u2t
                nsb = sb.tile([P, NC], F32, tag=f"nsb{b}{pti}")
                nc.vector.tensor_copy(out=nsb, in_=n_ps)
                nps[b][pti] = nsb

        # ---- stage 4: A = maskl * N * exp(-lnD); out = A.T @ X ----
        for b in range(BPC):
            a = []
            for pti in range(2):
                rd = sb.tile([P, NC], F32, tag=f"rd{b}{pti}")
                nc.scalar.activation(out=rd, in_=u2[b][pti], func=AF.Exp,
                                     scale=-1.0)
                a1 = sb.tile([P, NC], F32, tag=f"a1{b}{pti}")
                nc.vector.tensor_mul(out=a1, in0=nps[b][pti], in1=rd)
                a2 = sb.tile([P, NC], F32, tag=f"a2{b}{pti}")
                nc.gpsimd.tensor_mul(out=a2, in0=a1, in1=ml[pti])
                a.append(a2)
            for nt, (n0, nsz) in enumerate(((0, P), (P, NC - P))):
                o = pg_pool.tile([P, C], F32, tag=f"g{nt}")
                nc.tensor.matmul(out=o[:nsz, :], lhsT=a[0][:, n0:n0 + nsz],
                                 rhs=xt[b][0], start=True, stop=False)
                nc.tensor.matmul(out=o[:nsz, :], lhsT=a[1][:, n0:n0 + nsz],
                                 rhs=xt[b][1], start=False, stop=True)
                osb = sb.tile([P, C], BF16, tag=f"osb{b}{nt}")
                nc.vector.tensor_copy(out=osb[:nsz, :], in_=o[:nsz, :])
                nc.sync.dma_start(out=y[b, n0:n0 + nsz, :], in_=osb[:nsz, :])

    nc.compile()
    return nc


_ORIG_RUN_VIA_PJRT = _b2j.run_bass_via_pjrt
_FAST_CACHE = {}


def _build_fast_entry(nc, n_cores):
    """Reusable jitted executor for ``nc``.

    ``run_bass_via_pjrt`` builds a fresh jit closure per call, so every call
    re-traces, re-lowers, reloads the executable, and ships an extra zeroed
    donation buffer per output.  This kernel writes every element of its
    single output, so the zero-donation is unnecessary; caching the jitted
    callable turns repeat calls into pure transfer+execute."""
    from jax.experimental.shard_map import shard_map
    from jax.sharding import Mesh, PartitionSpec

    _b2j.install_neuronx_cc_hook()
    assert nc.dbg_addr is None and not nc.dbg_callbacks

    partition_name = nc.partition_id_tensor.name if nc.partition_id_tensor else None
    in_names = []
    out_names = []
    out_avals = []
    for alloc in nc.m.functions[0].allocations:
        if not isinstance(alloc, mybir.MemoryLocationSet):
            continue
        name = alloc.memorylocations[0].name
        if alloc.kind == "ExternalInput":
            if name != partition_name:
                in_names.append(name)
        elif alloc.kind == "ExternalOutput":
            out_names.append(name)
            out_avals.append(jax.core.ShapedArray(
                tuple(alloc.tensor_shape), mybir.dt.np(alloc.dtype)))
    n_params = len(in_names)
    bind_names = tuple(in_names) + ((partition_name,) if partition_name else ())

    def _body(*args):
        operands = list(args)
        if partition_name is not None:
            operands.append(_b2j.partition_id_tensor())
        return tuple(_b2j._bass_exec_p.bind(
            *operands,
            out_avals=tuple(out_avals),
            in_names=bind_names,
            out_names=tuple(out_names),
            lowering_input_output_aliases=(),
            sim_require_finite=True,
            sim_require_nnan=True,
            nc=nc,
        ))

    devices = jax.devices()[:n_cores]
    assert len(devices) == n_cores
    mesh = Mesh(np.asarray(devices), ("core",))
    fn = jax.jit(
        shard_map(_body, mesh=mesh,
                  in_specs=(PartitionSpec("core"),) * n_params,
                  out_specs=(PartitionSpec("core"),) * len(out_names),
                  check_rep=False),
        keep_unused=True,
    )

    def run(in_maps):
        per_core = [[np.asarray(m[name]) for name in in_names] for m in in_maps]
        concat = [
            np.concatenate([per_core[c][i] for c in range(n_cores)], axis=0)
            for i in range(n_params)
        ]
        out_arrs = fn(*concat)
        outs_np = [np.asarray(a) for a in out_arrs]
        return [
            {name: outs_np[i].reshape(n_cores, *out_avals[i].shape)[c]
             for i, name in enumerate(out_names)}
            for c in range(n_cores)
        ]

    return run


def _fast_run_via_pjrt(nc, in_maps, n_cores):
    try:
        run = _FAST_CACHE.get((id(nc), n_cores))
        if run is None:
            run = _build_fast_entry(nc, n_cores)
            _FAST_CACHE[(id(nc), n_cores)] = run
        return run(in_maps)
    except Exception:
        return _ORIG_RUN_VIA_PJRT(nc, in_maps, n_cores)


if _b2j.run_bass_via_pjrt is _ORIG_RUN_VIA_PJRT:
    _b2j.run_bass_via_pjrt = _fast_run_via_pjrt


_NC_CACHE = None


def _get_nc():
    global _NC_CACHE
    if _NC_CACHE is None:
        _NC_CACHE = build_bass()
    return _NC_CACHE


def build_in_maps(batch: np.ndarray, Wg: np.ndarray, bg: np.ndarray):
    X = np.ascontiguousarray(
        np.asarray(batch).reshape(B, HW, C).astype(ml_dtypes.bfloat16))
    wgb = np.concatenate([
        np.asarray(Wg, np.float32).reshape(C), np.asarray(bg, np.float32)])
    return [
        {"x": X[c * BPC:(c + 1) * BPC], "wgb": wgb}
        for c in range(NCORES)
    ]


def kernel(batch: np.ndarray, Wg: np.ndarray, bg: np.ndarray) -> np.ndarray:
    nc = _get_nc()
    in_maps = build_in_maps(batch, Wg, bg)
    res = run_bass_kernel_spmd(nc, in_maps, list(range(NCORES)))
    out = np.concatenate([np.asarray(res.results[c]["y"]) for c in range(NCORES)], 0)
    return out.reshape(B, CH, CW, C).astype(np.float32)
